# revision 29
# baseline (speedup 1.0000x reference)
"""Deformable conv (nn_DeformConv_31267361915085) Trainium2 Bass kernel, v3.

Sharding: data-parallel over (batch, H-half): core n handles batch n//2,
output rows [28*(n%2), 28*(n%2)+28). Weights replicated. SPMD: one program;
per-core input slabs are pre-shifted on host so the program is core-agnostic.

v3 (vs the v2 baseline, 160.8us -> ~138-141us):
  - Pipelined front-end: offset conv + coordinate math + index fold + SWDGE
    gathers run in 2 chunks of blocks (6/7) with per-3/4-block folds, so the
    first gather starts ~26us into the kernel instead of ~54us and chunk-B
    gathers overlap the first lerp blocks.
  - SWDGE warm-up: 4 tiny dummy gathers absorb the ~13us Q7 library/queue
    bring-up before the real gathers; num_idxs registers hoisted.
  - Quad table shrunk 72->46 rows (3.4MB vs 5.3MB input staging).
  - xcf loaded in 4 row-group DMAs (conv group 0 starts before full load);
    woff/boff loaded first (conv weights gate the first matmul).
  - 16 PE warm-up matmuls keep the clock ramped through the xcf wait (the
    offset conv then streams at ~168ns/matmul instead of ~330ns).
  - Index fold: per-subchunk DRAM roundtrip + doubling-ladder replicate
    (3 DMAs instead of 7; the shared HW DMA queue charges ~0.6us per DMA).
  - baseY/baseX precomputed once; per-chunk coordinate chain is 11 DVE ops.
  - offT psum->sbuf copies on the Scalar engine; diag builds emitted ahead
    of consumption on DVE.
  - main conv per block (128-col streams) with per-(block,oh) psum banks.
"""

import sys

if "/opt/trn_rl_repo" not in sys.path:
    sys.path.insert(0, "/opt/trn_rl_repo")

import contextlib

import numpy as np
import ml_dtypes

import concourse.bass as bass
import concourse.tile as tile
from concourse import bacc, mybir
from concourse.bass_utils import run_bass_kernel_spmd
from concourse.masks import make_identity

F32 = mybir.dt.float32
BF16 = mybir.dt.bfloat16
F8E3 = mybir.dt.float8e3
I16 = mybir.dt.int16
I32 = mybir.dt.int32
AL = mybir.AluOpType
ACT = mybir.ActivationFunctionType

# problem dims
B, CIN, H, W = 4, 256, 56, 56
COUT = 256
KK = 9
MARG = 8                # gather pad margin (covers |offset| <= ~6)
WQ = H + 2 * MARG       # 72: quad-table x extent
HQROWS = 46             # quad-table rows kept: y0m = floor(py)+7 <= ~42 (+margin)
NQ = HQROWS * WQ        # 3312 quad rows
NROWS = 28              # output rows per core
NPIX = NROWS * W        # 1568
BLK = 128               # pixels per block (raster order)
NBLK = 13               # ceil(1568/128) = 12.25 -> 13 (last block 96 pad)
NSLOT = NBLK * BLK      # 1664
NIDX = KK * BLK         # 1152 gather indices per block
CHUNKS = [(0, 6), (6, 7)]  # (first block, nblocks)
CBASE = [0, 64]  # idxT16 partition base per chunk (quadrant-aligned)
SUBF = [(0, 0, 3), (0, 3, 3), (1, 6, 4), (1, 10, 3)]  # (chunk, b0, nb) fold groups

_CACHE = {}


def _ap(base, offset_elems, dims):
    """AP with explicit free dims on top of a tile's base AP."""
    return bass.AP(
        tensor=base.tensor, offset=base.offset + offset_elems, ap=[base.ap[0]] + dims
    )


def build_nc():
    nc = bacc.Bacc(None, target_bir_lowering=False, num_swdge_queues=4)

    xcf_d = nc.dram_tensor("xcf", [128, 2, 30 * 58], BF16, kind="ExternalInput")
    xq_d = nc.dram_tensor("xq", [NQ, 1024], F8E3, kind="ExternalInput")
    woff_d = nc.dram_tensor("woff", [128, 2, KK, 18], BF16, kind="ExternalInput")
    boff_d = nc.dram_tensor("boff", [18, 1], F32, kind="ExternalInput")
    wm_d = nc.dram_tensor("wm", [128, KK, 2, 2, 128], BF16, kind="ExternalInput")
    out_d = nc.dram_tensor("out", [128, 2, NSLOT], F32, kind="ExternalOutput")
    idxd = nc.dram_tensor("idxd", [117 * 128], I16, kind="Internal")

    with tile.TileContext(nc) as tc, contextlib.ExitStack() as ctx:
        singles = ctx.enter_context(tc.tile_pool(name="singles", bufs=1))
        coords = ctx.enter_context(tc.tile_pool(name="coords", bufs=1))
        gp = ctx.enter_context(tc.tile_pool(name="gp", bufs=10))
        dp = ctx.enter_context(tc.tile_pool(name="dp", bufs=3))
        rp = ctx.enter_context(tc.tile_pool(name="rp", bufs=2))
        osb = ctx.enter_context(tc.tile_pool(name="osb", bufs=4))
        mainp = ctx.enter_context(tc.tile_pool(name="mainp", bufs=2, space="PSUM"))
        fep = ctx.enter_context(tc.tile_pool(name="fep", bufs=1, space="PSUM"))
        oup = ctx.enter_context(tc.tile_pool(name="oup", bufs=2, space="PSUM"))
        kap = ctx.enter_context(tc.tile_pool(name="kap", bufs=1, space="PSUM"))

        # ---- input loads (sync DMA queue; conv-critical tensors first) ----
        woff = singles.tile([128, 2, KK, 18], BF16)
        nc.sync.dma_start(out=woff[:, :, :, :], in_=woff_d[:, :, :, :])
        boff = singles.tile([18, 1], F32)
        nc.sync.dma_start(out=boff[:, :], in_=boff_d[:, :])
        xcf = singles.tile([128, 2, 30 * 58], BF16)
        for (ra, rb) in ((0, 10), (10, 17), (17, 24), (24, 30)):
            nc.sync.dma_start(
                out=xcf[:, :, ra * 58 : rb * 58], in_=xcf_d[:, :, ra * 58 : rb * 58]
            )
        wm = singles.tile([128, KK, 2, 2, 128], BF16)
        nc.sync.dma_start(out=wm[:, :, :, :, :], in_=wm_d[:, :, :, :, :])

        # ---- gpsimd early: ident/iotas, then SWDGE warm-up (Q7 bring-up ~13us) ----
        ident_f = singles.tile([128, 128], F32)
        make_identity(nc, ident_f[:, :])

        p_i = coords.tile([128, NBLK], I32)
        nc.gpsimd.iota(p_i[:, :], pattern=[[BLK, NBLK]], base=0, channel_multiplier=1)
        kyM_i = coords.tile([128, KK], I32)
        nc.gpsimd.iota(
            kyM_i[:, :], pattern=[[1, 3], [0, 3]], base=MARG - 1, channel_multiplier=0
        )
        kxM_i = coords.tile([128, KK], I32)
        nc.gpsimd.iota(
            kxM_i[:, :], pattern=[[0, 3], [1, 3]], base=MARG - 1, channel_multiplier=0
        )

        idx_dummy = singles.tile([128, 8], I16)
        nc.gpsimd.memset(idx_dummy[:, :], 0.0)
        g_dummy = singles.tile([128, 4, 1, 1024], F8E3)
        r_full = nc.alloc_register(mybir.EngineType.Pool, "nidx_full")
        nc.reg_mov(r_full, NIDX)
        r_split = nc.alloc_register(mybir.EngineType.Pool, "nidx_split")
        nc.reg_mov(r_split, NIDX // 3)
        for q in range(4):
            nc.gpsimd.dma_gather(
                out_ap=g_dummy[:, q, :, :],
                in_ap=xq_d[:, :],
                idxs_ap=idx_dummy[:, 0:1],
                num_idxs=16,
                num_idxs_reg=16,
                elem_size=1024,
                single_packet=False,
                queue_num=q,
            )

        # ---- vector pre-compute (independent of the offset conv) ----
        ident_b = singles.tile([128, 128], BF16)
        nc.vector.tensor_copy(out=ident_b[:, :], in_=ident_f[:, :])

        off_sb = coords.tile([18, NSLOT], F32)
        nc.vector.memset(off_sb[:, NPIX:NSLOT], 0.0)

        p_f = coords.tile([128, NBLK], F32)
        nc.vector.tensor_copy(out=p_f[:, :], in_=p_i[:, :])
        t56 = coords.tile([128, NBLK], F32)
        nc.vector.tensor_scalar(
            out=t56[:, :], in0=p_f[:, :], scalar1=0.5, scalar2=1.0 / 56.0,
            op0=AL.add, op1=AL.mult,
        )
        _fc = [0]

        def floor_fix(dst_f, src, shape):
            """dst_f = floor(src) for src >= 0 (i32 round-to-nearest + fixup).
            dst_f/src are APs (or tiles) of `shape`; scratch tiles per call."""
            _fc[0] += 1
            sl = (slice(None),) * len(shape)
            if not isinstance(dst_f, bass.AP):
                dst_f = dst_f[sl]
            if not isinstance(src, bass.AP):
                src = src[sl]
            ci = coords.tile(shape, I32, name=f"ci{_fc[0]}")
            nc.vector.tensor_copy(out=ci[sl], in_=src)
            nc.vector.tensor_copy(out=dst_f, in_=ci[sl])
            gt = coords.tile(shape, F32, name=f"gt{_fc[0]}")
            nc.vector.tensor_tensor(
                out=gt[sl], in0=dst_f, in1=src, op=AL.is_gt
            )
            nc.vector.tensor_tensor(
                out=dst_f, in0=dst_f, in1=gt[sl], op=AL.subtract
            )

        r_f = coords.tile([128, NBLK], F32)
        floor_fix(r_f, t56, [128, NBLK])
        jx = coords.tile([128, NBLK], F32)
        nc.vector.scalar_tensor_tensor(
            out=jx[:, :], in0=r_f[:, :], scalar=-56.0, in1=p_f[:, :],
            op0=AL.mult, op1=AL.add,
        )
        kyM = coords.tile([128, KK], F32)
        nc.vector.tensor_copy(out=kyM[:, :], in_=kyM_i[:, :])
        kxM = coords.tile([128, KK], F32)
        nc.vector.tensor_copy(out=kxM[:, :], in_=kxM_i[:, :])

        # baseY/baseX = broadcast(r/jx over k) + kyM/kxM, built once.
        P3 = [128, NBLK, KK]
        baseY = coords.tile(P3, F32)
        baseX = coords.tile(P3, F32)
        kyM_b = _ap(kyM[:], 0, [[0, NBLK], [1, KK]])
        kxM_b = _ap(kxM[:], 0, [[0, NBLK], [1, KK]])
        # copy walks (k outer, bb inner) so the stride-0 broadcast dim is outer
        nc.vector.tensor_copy(
            out=_ap(baseY[:], 0, [[1, KK], [KK, NBLK]]),
            in_=_ap(r_f[:], 0, [[0, KK], [1, NBLK]]),
        )
        nc.vector.tensor_tensor(
            out=baseY[:, :, :], in0=baseY[:, :, :], in1=kyM_b, op=AL.add
        )
        nc.vector.tensor_copy(
            out=_ap(baseX[:], 0, [[1, KK], [KK, NBLK]]),
            in_=_ap(jx[:], 0, [[0, KK], [1, NBLK]]),
        )
        nc.vector.tensor_tensor(
            out=baseX[:, :, :], in0=baseX[:, :, :], in1=kxM_b, op=AL.add
        )

        # full-size coordinate tiles (written per chunk)
        offT = coords.tile([128, NBLK, KK * 2], F32)
        pym = coords.tile(P3, F32)
        pxm = coords.tile(P3, F32)
        ty = coords.tile(P3, F32)
        tx = coords.tile(P3, F32)
        y0 = coords.tile(P3, F32)
        x0 = coords.tile(P3, F32)
        idxf = coords.tile(P3, F32)
        u = coords.tile(P3, F32)
        v = coords.tile(P3, F32)
        alphas = coords.tile([128, 4, NBLK, KK], F32)
        ab16 = coords.tile([128, NBLK, 4, KK], BF16)
        adup = coords.tile([128, NBLK * 36, 2], BF16)
        idxT16 = coords.tile([128, 16, 8], I16)  # chunk c rows at 32c (quadrant-aligned)
        idxw = coords.tile([128, NBLK, 72], I16)
        ppw = idxw[:, :, :].ap[0][0]

        # ---- PE warm-up: ramp the clock before the offset conv ----
        wps = fep.tile([128, 128], F32, tag="fe", name="wps")
        for _ in range(16):
            nc.tensor.matmul(
                wps[:, :], ident_f[:, :], ident_f[:, :], start=True, stop=True
            )

        # ================= front-end emission helpers =================
        def conv_ns(ns):
            """offset conv for output rows [7ns, 7ns+7): psum [18, 392]."""
            ps_o = mainp.tile([18, 392], F32, tag="pt", name=f"pso{ns}")
            for kc in range(18):
                k, ch = divmod(kc, 2)
                ky, kx = divmod(k, 3)
                rhs = _ap(
                    xcf[:, :, :],
                    ch * 1740 + (ns * 7 + ky) * 58 + kx,
                    [[58, 7], [1, 56]],
                )
                nc.tensor.matmul(
                    ps_o[:, :],
                    woff[:, ch, k, :],
                    rhs,
                    start=(kc == 0),
                    stop=(kc == 17),
                )
            nc.vector.tensor_scalar(
                out=off_sb[:, ns * 392 : (ns + 1) * 392],
                in0=ps_o[:, :],
                scalar1=boff[:, 0:1],
                scalar2=None,
                op0=AL.add,
            )

        ps_ts = {}

        def chunk_T(ci):
            """PE transposes of off_sb columns for the chunk's blocks."""
            b0, nb = CHUNKS[ci]
            ps_t = fep.tile([128, nb, 18], F32, tag="fe", name=f"pst{ci}")
            ps_ts[ci] = ps_t
            for i in range(nb):
                nc.tensor.transpose(
                    ps_t[:, i, :],
                    off_sb[:18, (b0 + i) * BLK : (b0 + i + 1) * BLK],
                    ident_f[:18, :18],
                )

        def chunk_cp(ci):
            b0, nb = CHUNKS[ci]
            nc.scalar.copy(
                out=_ap(offT[:, :, :], b0 * 18, [[18, nb], [1, 18]]),
                in_=ps_ts[ci][:, :, :],
            )

        def chunk_idx(ci):
            """DVE coordinate chain for the chunk's blocks -> idxf."""
            b0, nb = CHUNKS[ci]
            sl = (slice(None), slice(b0, b0 + nb), slice(None))
            dy = _ap(offT[:, :, :], b0 * 18, [[18, nb], [2, KK]])
            dx = _ap(offT[:, :, :], b0 * 18 + 1, [[18, nb], [2, KK]])
            nc.vector.tensor_tensor(out=pym[sl], in0=dy, in1=baseY[sl], op=AL.add)
            nc.vector.tensor_tensor(out=pxm[sl], in0=dx, in1=baseX[sl], op=AL.add)
            shp = [128, nb, KK]
            y0s = bass.AP(tensor=y0.tensor, offset=y0.offset + b0 * KK,
                          ap=[y0.ap[0], [KK, nb], [1, KK]])
            x0s = bass.AP(tensor=x0.tensor, offset=x0.offset + b0 * KK,
                          ap=[x0.ap[0], [KK, nb], [1, KK]])
            floor_fix(y0s, pym[sl], shp)
            floor_fix(x0s, pxm[sl], shp)
            nc.vector.scalar_tensor_tensor(
                out=idxf[sl], in0=y0[sl], scalar=float(WQ), in1=x0[sl],
                op0=AL.mult, op1=AL.add,
            )

        ps_is = {}

        def chunk_idxT(ci):
            b0, nb = CHUNKS[ci]
            ps_i = fep.tile([nb * KK, 128], F32, tag="fe", name=f"psi{ci}")
            ps_is[ci] = ps_i
            nc.tensor.transpose(
                ps_i[:, :],
                _ap(idxf[:, :, :], b0 * KK, [[1, nb * KK]]),
                ident_f[:, :],
            )

        def chunk_perm(ci):
            b0, nb = CHUNKS[ci]
            base = CBASE[ci]
            # permute columns to wrap order: dst col q*8+t <- pixel 16t+q
            nc.vector.tensor_copy(
                out=idxT16[base : base + nb * KK, :, :],
                in_=_ap(ps_is[ci][:, :], 0, [[1, 16], [16, 8]]),
            )

        def sub_fold(ci, b0, nb):
            """v2-style fold: 256B-row DMA to DRAM, one 16B-element permuting
            DMA back, then a doubling-ladder replicate."""
            rbase = CBASE[ci] + (b0 - CHUNKS[ci][0]) * KK
            nc.sync.dma_start(
                out=bass.AP(
                    tensor=idxd, offset=b0 * KK * 128, ap=[[128, nb * KK], [1, 128]]
                ),
                in_=idxT16[rbase : rbase + nb * KK, :, :],
            )
            nc.scalar.dma_start(
                out=bass.AP(
                    tensor=idxw.tensor,
                    offset=idxw.offset + b0 * 72,
                    ap=[[ppw, 16], [72, nb], [8, KK], [1, 8]],
                ),
                in_=bass.AP(
                    tensor=idxd,
                    offset=b0 * KK * 128,
                    ap=[[8, 16], [KK * 128, nb], [128, KK], [1, 8]],
                ),
            )
            # doubling ladder: group 0 -> 1, groups 0-1 -> 2-3, 0-3 -> 4-7
            for np_, eng in ((16, nc.sync), (32, nc.scalar), (64, nc.sync)):
                eng.dma_start(
                    out=bass.AP(
                        tensor=idxw.tensor,
                        offset=idxw.offset + np_ * ppw + b0 * 72,
                        ap=[[ppw, np_], [1, nb * 72]],
                    ),
                    in_=bass.AP(
                        tensor=idxw.tensor,
                        offset=idxw.offset + b0 * 72,
                        ap=[[ppw, np_], [1, nb * 72]],
                    ),
                )

        g_tiles = {}

        def sub_gather(ci, b0, nb):
            for bb in range(b0, b0 + nb):
                g = gp.tile([128, KK, 1024], F8E3, tag="g", name=f"g{bb}")
                g_tiles[bb] = g
                if bb < 3:
                    # pipeline fill: split into 3-tap sub-gathers on separate
                    # queues; finer completion granularity lets the first
                    # lerp groups start on partial data
                    for j in range(3):
                        nc.gpsimd.dma_gather(
                            out_ap=g[:, 3 * j : 3 * j + 3, :],
                            in_ap=xq_d[:, :],
                            idxs_ap=idxw[:, bb, 24 * j : 24 * j + 24],
                            num_idxs=NIDX // 3,
                            num_idxs_reg=r_split,
                            elem_size=1024,
                            single_packet=False,
                            queue_num=(bb * 3 + j) % 4,
                        )
                else:
                    nc.gpsimd.dma_gather(
                        out_ap=g[:, :, :],
                        in_ap=xq_d[:, :],
                        idxs_ap=idxw[:, bb, :],
                        num_idxs=NIDX,
                        num_idxs_reg=r_full,
                        elem_size=1024,
                        single_packet=False,
                        queue_num=bb % 4,
                    )

        def chunk_alpha(ci):
            b0, nb = CHUNKS[ci]
            sl = (slice(None), slice(b0, b0 + nb), slice(None))
            nc.vector.tensor_tensor(
                out=ty[sl], in0=pym[sl], in1=y0[sl], op=AL.subtract
            )
            nc.vector.tensor_tensor(
                out=tx[sl], in0=pxm[sl], in1=x0[sl], op=AL.subtract
            )
            nc.vector.tensor_scalar(
                out=u[sl], in0=tx[sl], scalar1=-1.0, scalar2=1.0,
                op0=AL.mult, op1=AL.add,
            )
            nc.vector.tensor_scalar(
                out=v[sl], in0=ty[sl], scalar1=-1.0, scalar2=1.0,
                op0=AL.mult, op1=AL.add,
            )
            for q, (fy, fx_) in enumerate(((v, u), (v, tx), (ty, u), (ty, tx))):
                nc.vector.tensor_tensor(
                    out=alphas[:, q, b0 : b0 + nb, :],
                    in0=fy[sl],
                    in1=fx_[sl],
                    op=AL.mult,
                )
            nc.vector.tensor_copy(
                out=ab16[:, b0 : b0 + nb, :, :],
                in_=_ap(
                    alphas[:, :, :, :], b0 * KK,
                    [[KK, nb], [NBLK * KK, 4], [1, KK]],
                ),
            )
            nc.vector.tensor_copy(
                out=adup[:, b0 * 36 : (b0 + nb) * 36, :],
                in_=_ap(ab16[:, :, :, :], b0 * 36, [[1, nb * 36], [0, 2]]),
            )

        diag_tiles = {}

        def diag_build(bb):
            # diag[p, kq, j] = ident[p, j] * alpha[p, kq]; duplicated-pair APs
            # keep innermost strides 1 -> DVE 2x mode
            diag = dp.tile([128, 36, 128], BF16, tag="diag", name=f"diag{bb}")
            diag_tiles[bb] = diag
            nc.vector.tensor_tensor(
                out=_ap(diag[:, :, :], 0, [[128, 36], [2, 64], [1, 2]]),
                in0=_ap(ident_b[:, :], 0, [[0, 36], [2, 64], [1, 2]]),
                in1=_ap(adup[:, :, :], bb * 72, [[2, 36], [0, 64], [1, 2]]),
                op=AL.mult,
            )

        def consumer(bb):
            """lerp-transposes + per-block main conv + output DMA."""
            g = g_tiles[bb]
            diag = diag_tiles[bb]
            ncols = 32 if bb == NBLK - 1 else 128  # last block: 96 pad columns
            rhs_t = rp.tile([128, KK, 2, 128], BF16, tag="rhs", name=f"rhs{bb}")

            def lerp_grp(grp):
                pt = mainp.tile([128, 3, 2, 128], F32, tag="pt", name=f"pt{bb}_{grp}")
                for kk in range(3):
                    k = grp * 3 + kk
                    for ch in range(2):
                        for q in range(4):
                            nc.tensor.matmul(
                                pt[:, kk, ch, 0:ncols],
                                _ap(g[:, :, :], k * 1024 + q * 256 + ch * 128, [[1, 128]]),
                                diag[:, q * KK + k, 0:ncols],
                                start=(q == 0),
                                stop=(q == 3),
                            )
                nc.scalar.copy(
                    out=_ap(
                        rhs_t[:, :, :, :], grp * 3 * 256,
                        [[256, 3], [128, 2], [1, ncols]],
                    ),
                    in_=_ap(pt[:, :, :, :], 0, [[256, 3], [128, 2], [1, ncols]]),
                )

            outps = [
                oup.tile([128, 128], F32, tag="outp", name=f"op{bb}_{oh}")
                for oh in range(2)
            ]

            def mc(oh, kc0, kc1):
                for kc in range(kc0, kc1):
                    k, ch = divmod(kc, 2)
                    nc.tensor.matmul(
                        outps[oh][:, 0:ncols],
                        wm[:, k, ch, oh, :],
                        _ap(rhs_t[:, :, :, :], (k * 2 + ch) * 128, [[1, ncols]]),
                        start=(kc == kc0 == 0),
                        stop=(kc == 17),
                    )

            lerp_grp(0)
            lerp_grp(1)
            lerp_grp(2)
            mc(0, 0, 18)
            mc(1, 0, 18)
            for oh in range(2):
                o_t = osb.tile([128, 128], F32, tag="ot", name=f"ot{bb}_{oh}")
                nc.scalar.copy(out=o_t[:, 0:ncols], in_=outps[oh][:, 0:ncols])
                nc.sync.dma_start(
                    out=out_d[:, oh, bb * BLK : bb * BLK + ncols],
                    in_=o_t[:, 0:ncols],
                )

        # ================= the pipelined program =================
        conv_ns(0)
        conv_ns(1)
        chunk_T(0); chunk_cp(0); chunk_idx(0); chunk_idxT(0); chunk_perm(0)
        sub_fold(0, 0, 3); sub_gather(0, 0, 3); sub_fold(0, 3, 3); sub_gather(0, 3, 3)
        conv_ns(2)
        conv_ns(3)
        chunk_T(1); chunk_cp(1); chunk_idx(1); chunk_idxT(1); chunk_perm(1)
        sub_fold(1, 6, 4); sub_gather(1, 6, 4); sub_fold(1, 10, 3); sub_gather(1, 10, 3)
        chunk_alpha(0)
        diag_build(0); diag_build(1); diag_build(2)
        chunk_alpha(1)
        diag_build(3); diag_build(4)
        for bb in range(NBLK):
            consumer(bb)
            if bb + 5 < NBLK:
                diag_build(bb + 5)

        # PE keep-alive: zero-dependency matmuls emitted at lowest priority.
        # The list scheduler only slots them where the PE would otherwise
        # idle (waiting on DVE/gathers), which keeps the DVFS clock ramped
        # so real matmuls after a stall run at full rate instead of ~2x slow.
        wka = kap.tile([128, 128], F32, tag="ka", name="wka")
        for _ in range(80):
            nc.tensor.matmul(
                wka[:, :], ident_b[:, :], ident_b[:, :], start=True, stop=True
            )

    nc.compile()
    return nc


def prep_inputs(x, w_off, b_off, w):
    """Host-side slab/layout prep. Returns list of 8 per-core input dicts."""
    x = np.asarray(x, dtype=np.float32)
    w_off = np.asarray(w_off, dtype=np.float32)
    b_off = np.asarray(b_off, dtype=np.float32)
    w = np.asarray(w, dtype=np.float32)

    woff_arr = np.ascontiguousarray(
        w_off.reshape(18, 2, 128, KK).transpose(2, 1, 3, 0)
    ).astype(ml_dtypes.bfloat16)  # [128 cl, 2 ch, 9 k, 18 o]
    boff_arr = np.ascontiguousarray(b_off.reshape(18, 1))
    wm_arr = np.ascontiguousarray(
        w.reshape(2, 128, 2, 128, KK).transpose(3, 4, 2, 0, 1)
    ).astype(ml_dtypes.bfloat16)  # [128 cl, 9 k, 2 ch, 2 ot, 128 ol]

    in_maps = []
    for core in range(8):
        b, half = divmod(core, 2)
        r0 = half * NROWS
        xb = x[b]  # [256, 56, 56]

        xp58 = np.zeros((CIN, 58, 58), np.float32)
        xp58[:, 1:57, 1:57] = xb
        xcf = np.ascontiguousarray(
            xp58[:, r0 : r0 + 30, :].reshape(2, 128, 30 * 58).transpose(1, 0, 2)
        ).astype(ml_dtypes.bfloat16)

        xp = np.zeros((HQROWS + 1, WQ + 1, CIN), np.float32)
        ylo = max(0, r0 - MARG)
        yhi = min(H, r0 + HQROWS + 1 - MARG)
        xhwc = xb.transpose(1, 2, 0)
        xp[ylo - (r0 - MARG) : yhi - (r0 - MARG), MARG : MARG + W, :] = xhwc[ylo:yhi]
        quad = np.stack(
            [xp[:-1, :-1], xp[:-1, 1:], xp[1:, :-1], xp[1:, 1:]], axis=2
        )  # [46, 72, 4, 256]
        xq = np.ascontiguousarray(quad.reshape(NQ, 4 * CIN)).astype(
            ml_dtypes.float8_e3m4
        )

        in_maps.append(
            {
                "xcf": xcf,
                "woff": woff_arr,
                "boff": boff_arr,
                "wm": wm_arr,
                "xq": xq,
            }
        )
    return in_maps


def unshard_output(results):
    """results: list of 8 per-core out arrays [128, 2, NSLOT] -> [B,COUT,H,W]."""
    out = np.zeros((B, COUT, H, W), np.float32)
    for core in range(8):
        b, half = divmod(core, 2)
        r0 = half * NROWS
        oc = results[core]  # [128 ol, 2 oh, NSLOT]
        oc = oc.transpose(1, 0, 2).reshape(COUT, NSLOT)[:, :NPIX]
        out[b, :, r0 : r0 + NROWS, :] = oc.reshape(COUT, NROWS, W)
    return out


def kernel(**inputs):
    nc = _CACHE.get("nc")
    if nc is None:
        nc = build_nc()
        _CACHE["nc"] = nc
    in_maps = prep_inputs(
        inputs["x"], inputs["w_off"], inputs["b_off"], inputs["w"]
    )
    res = run_bass_kernel_spmd(nc, in_maps, core_ids=list(range(8)))
    return unshard_output([r["out"] for r in res.results])
